# revision 23
# baseline (speedup 1.0000x reference)
"""Trainium2 Bass kernel for CustomLoss:
    out = mean_{b,t} CE(logits[b,t,:], tgt[b,t]) + penalty
    CE   = logsumexp_V(logits) - logits[tgt]
    penalty = sum_b C(n_b, 2), n_b = #{t : sizes[b, argmax_V logits[b,t,:]] > 0}

Sharding: data-parallel over the 4096 (b,t) tokens -> 512 tokens/core on 8
NeuronCores. Each core streams its [512, 32000] logits shard through SBUF
once at HBM rate; ACT computes exp into a bf16 copy with fused fp32 row-sum
accumulation (logsumexp), DVE computes per-block maxes over the bf16 exp
copy (exp is monotonic, so the argmax block is unchanged) with a chain of
16-bit tensor_tensor MAX folds that run at 2 elem/cycle -- roughly half the
cost of a direct fp32 tensor_reduce, keeping DVE well below the DMA stream
even when the engine clocks are throttled. The winning 256-wide block's
logits and sizes are gathered concurrently per tile; all gather-consuming
ops are deferred until the end of the stream so SWDGE gather latency never
stalls the in-order DVE queue. The final tile ends with a tiny 1024-column
chunk so the post-stream critical path is short. Per-core partial sums
leave as a [128, 2] tile; partitions are summed on host.
"""

from contextlib import ExitStack

import numpy as np

P = 128
V = 32000
B, T = 2, 2048
N_CORES = 8
TOK = (B * T) // N_CORES      # 512 tokens per core
NT = TOK // P                 # 4 token tiles of 128 partitions
W = 256                       # argmax block width
NB = V // W                   # 125 blocks per token row
VC = 6400                     # default vocab chunk (25 blocks)
NBC = VC // W                 # blocks per full chunk
# per-tile (vocab_offset, width) chunk lists; the last tile ends with a tiny
# chunk so the tail reduce after the DMA stream drains is short
_FULL = [(k * VC, VC) for k in range(V // VC)]
# geometric taper: each chunk's exp (0.83 ns/col) fits under the next
# chunk's DMA transfer (>=1.2 ns/col), so ACT stays caught up to the end
_LAST = _FULL[:3] + [(19200, 4096), (23296, 3072), (26368, 2304),
                     (28672, 1536), (30208, 1024), (31232, 768)]
CHUNKS = [_FULL, _FULL, _FULL, _LAST]
MAXCH = max(len(c) for c in CHUNKS)
ALPHA = 1.0

_NC_CACHE = {}


def _build_nc():
    """Build the single-core Bass program (identical on all 8 cores)."""
    import concourse.bacc as bacc
    import concourse.bass as bass
    import concourse.mybir as mybir
    import concourse.tile as tile

    f32 = mybir.dt.float32
    bf16 = mybir.dt.bfloat16
    i32 = mybir.dt.int32
    u32 = mybir.dt.uint32
    AF = mybir.ActivationFunctionType
    ALU = mybir.AluOpType
    AX = mybir.AxisListType

    nc = bacc.Bacc("TRN2", target_bir_lowering=False)
    logits = nc.declare_dram_parameter("logits", [TOK, V], f32, isOutput=False)
    # flat element index t*V + tgt[t], laid out [p, tile] (token = tt*128 + p)
    tgt_off = nc.declare_dram_parameter("tgt_off", [P, NT], i32, isOutput=False)
    sizes_r = nc.declare_dram_parameter("sizes_r", [NB, W], f32, isOutput=False)
    out = nc.declare_dram_parameter("out", [P, 2], f32, isOutput=True)

    with tile.TileContext(nc) as tc, ExitStack() as ctx:
        lp = ctx.enter_context(tc.tile_pool(name="lp", bufs=5))
        ep = ctx.enter_context(tc.tile_pool(name="ep", bufs=3))
        fp = ctx.enter_context(tc.tile_pool(name="fp", bufs=2))
        sm = ctx.enter_context(tc.tile_pool(name="sm", bufs=4))
        ph = ctx.enter_context(tc.tile_pool(name="ph", bufs=2))
        cst = ctx.enter_context(tc.tile_pool(name="cst", bufs=1))

        # ---- persistent constants / accumulators ----
        # row base (flat element index) for each (partition, tile):
        # rb[p, tt] = (tt*P + p) * V  -- exact in f32 (max < 2^24).
        # iota free-axis steps are int16-limited, so one iota per tile column.
        rb_i = cst.tile([P, NT], i32)
        for tt in range(NT):
            nc.gpsimd.iota(
                rb_i[:, tt : tt + 1], pattern=[[1, 1]], base=tt * P * V,
                channel_multiplier=V,
            )
        rb_f = cst.tile([P, NT], f32)
        nc.vector.tensor_copy(rb_f[:], rb_i[:])

        tgt_idx = cst.tile([P, NT], i32)
        # scalar (ACT) HWDGE queue, so the sync queue's first job is chunk 0
        nc.scalar.dma_start(tgt_idx[:], tgt_off[:, :])
        tgt_logit = cst.tile([P, NT], f32)
        for tt in range(NT):
            nc.gpsimd.indirect_dma_start(
                out=tgt_logit[:, tt : tt + 1],
                out_offset=None,
                in_=logits[:, :],
                in_offset=bass.IndirectOffsetOnAxis(
                    ap=tgt_idx[:, tt : tt + 1], axis=1
                ),
            )

        tot_cols = cst.tile([P, NT], f32)   # per-tile sum(exp) totals
        m_cols = cst.tile([P, NT], f32)     # per-tile positive-size indicator
        acc = cst.tile([P, 2], f32)
        # per-tile gather landing zones (consumed late, so the gathers'
        # HBM latency never stalls the in-order DVE queue mid-stream)
        szb_all = cst.tile([P, NT * W], f32)
        blk_all = cst.tile([P, NT * W], f32)

        def do_chunk(tt, bmax, sexp, c, off, vc):
            nb = vc // W
            lt = lp.tile([P, VC], f32, tag="lt")
            nc.sync.dma_start(
                lt[:, :vc], logits[tt * P : (tt + 1) * P, off : off + vc]
            )
            et = ep.tile([P, VC], bf16, tag="et")
            nc.scalar.activation(
                et[:, :vc], lt[:, :vc], AF.Exp, accum_out=sexp[:, c : c + 1]
            )
            # block-max over the bf16 exp copy: three 2x-rate TT-max folds
            # [P,nb,256] -> [P,nb,32], then a small 1x reduce to [P,nb].
            # Tiny chunks skip the folds: 4 instruction overheads cost more
            # wall time than a direct 1x reduce below ~1.5k columns.
            e3 = et[:, :vc].rearrange("p (b w) -> p b w", w=W)
            if vc <= 1536:
                nc.vector.tensor_reduce(
                    bmax[:, off // W : (off + vc) // W], e3, axis=AX.X,
                    op=ALU.max,
                )
                return
            f1 = fp.tile([P, NBC * 128], bf16, tag="f1")
            f13 = f1[:, : nb * 128].rearrange("p (b w) -> p b w", w=128)
            nc.vector.tensor_tensor(
                f13, e3[:, :, 0:128], e3[:, :, 128:256], op=ALU.max
            )
            f2 = fp.tile([P, NBC * 64], bf16, tag="f2")
            f23 = f2[:, : nb * 64].rearrange("p (b w) -> p b w", w=64)
            nc.vector.tensor_tensor(
                f23, f13[:, :, 0:64], f13[:, :, 64:128], op=ALU.max
            )
            f3 = fp.tile([P, NBC * 32], bf16, tag="f3")
            f33 = f3[:, : nb * 32].rearrange("p (b w) -> p b w", w=32)
            nc.vector.tensor_tensor(
                f33, f23[:, :, 0:32], f23[:, :, 32:64], op=ALU.max
            )
            nc.vector.tensor_reduce(
                bmax[:, off // W : (off + vc) // W], f33, axis=AX.X, op=ALU.max
            )

        def select_tile(tt, bmax, sexp, nch):
            """Pick the winning block and *issue* its two gathers (nothing
            here reads gathered data, so no gather-latency DVE stalls)."""
            top8 = sm.tile([P, 8], bf16, tag="top8")
            nc.vector.max(top8[:], bmax[:])
            bix8 = sm.tile([P, 8], u32, tag="bix8")
            nc.vector.max_index(bix8[:], top8[:], bmax[:])
            # logits-block gather first (its consumer chain is the longer
            # pole in the tail): flat elem idx = rb + bid*W, exact in f32
            bidf = sm.tile([P, 1], f32, tag="bidf")
            nc.vector.tensor_copy(bidf[:], bix8[:, 0:1])
            gsf = sm.tile([P, 1], f32, tag="gsf")
            nc.vector.tensor_scalar(
                gsf[:], bidf[:], float(W), rb_f[:, tt : tt + 1],
                op0=ALU.mult, op1=ALU.add,
            )
            gsi = sm.tile([P, 1], i32, tag="gsi")
            nc.vector.tensor_copy(gsi[:], gsf[:])
            nc.gpsimd.indirect_dma_start(
                out=blk_all[:, tt * W : (tt + 1) * W],
                out_offset=None,
                in_=logits[:, :],
                in_offset=bass.IndirectOffsetOnAxis(ap=gsi[:, 0:1], axis=1),
            )
            bid_i = sm.tile([P, 1], i32, tag="bid_i")
            nc.vector.tensor_copy(bid_i[:], bix8[:, 0:1])
            nc.gpsimd.indirect_dma_start(
                out=szb_all[:, tt * W : (tt + 1) * W],
                out_offset=None,
                in_=sizes_r[:, :],
                in_offset=bass.IndirectOffsetOnAxis(ap=bid_i[:, 0:1], axis=0),
            )
            # off the argmax critical path, so last: per-tile exp total
            nc.vector.reduce_sum(
                tot_cols[:, tt : tt + 1], sexp[:, :nch], axis=AX.X
            )

        def finish_tile(tt):
            """Consume the gathered blocks: size-positivity at the argmax.
            The one-hot comes from comparing the block against its own max
            (values are distinct floats), so no index math is needed."""
            blk = blk_all[:, tt * W : (tt + 1) * W]
            szb = szb_all[:, tt * W : (tt + 1) * W]
            # blk was gathered first, so consume it first
            blk8 = sm.tile([P, 8], f32, tag="blk8")
            nc.vector.max(blk8[:], blk)
            oh = ph.tile([P, W], f32, tag="oh")
            nc.vector.tensor_scalar(
                oh[:], blk, blk8[:, 0:1], None, op0=ALU.is_equal
            )
            mb = ph.tile([P, W], f32, tag="mb")
            nc.vector.tensor_scalar(mb[:], szb, 0.0, None, op0=ALU.is_gt)
            prod = ph.tile([P, W], f32, tag="prod")
            nc.vector.tensor_tensor(prod[:], oh[:], mb[:], op=ALU.mult)
            nc.vector.tensor_reduce(
                m_cols[:, tt : tt + 1], prod[:], axis=AX.X, op=ALU.add
            )

        for tt in range(NT - 1):
            bmax = sm.tile([P, NB], bf16, tag="bmax")
            sexp = sm.tile([P, MAXCH], f32, tag="sexp")
            for c, (off, vc) in enumerate(CHUNKS[tt]):
                do_chunk(tt, bmax, sexp, c, off, vc)
            select_tile(tt, bmax, sexp, len(CHUNKS[tt]))
        # last tile: stream its big chunks, slot the earlier tiles' deferred
        # work in while its data is still in flight, then its tiny tail
        # chunk, so the post-stream critical path is short
        last = NT - 1
        chunks = CHUNKS[last]
        bmax = sm.tile([P, NB], bf16, tag="bmax")
        sexp = sm.tile([P, MAXCH], f32, tag="sexp")
        for c, (off, vc) in enumerate(chunks[:3]):
            do_chunk(last, bmax, sexp, c, off, vc)
        for tt in range(NT - 1):
            finish_tile(tt)
        for c, (off, vc) in enumerate(chunks[3:], start=3):
            do_chunk(last, bmax, sexp, c, off, vc)
        select_tile(last, bmax, sexp, len(chunks))
        finish_tile(last)

        # ---- nll, batched: one Ln activation for all tiles ----
        lse_cols = cst.tile([P, NT], f32)
        nc.scalar.activation(lse_cols[:], tot_cols[:], AF.Ln)
        nll_cols = cst.tile([P, NT], f32)
        nc.vector.tensor_tensor(
            nll_cols[:], lse_cols[:], tgt_logit[:], op=ALU.subtract
        )
        nc.vector.reduce_sum(acc[:, 0:1], nll_cols[:], axis=AX.X)
        nc.vector.reduce_sum(acc[:, 1:2], m_cols[:], axis=AX.X)
        nc.sync.dma_start(out[:, :], acc[:])

    nc.finalize()
    return nc


def _get_nc():
    if "nc" not in _NC_CACHE:
        _NC_CACHE["nc"] = _build_nc()
    return _NC_CACHE["nc"]


def _make_in_maps(logits, tgt, sizes):
    logits = np.ascontiguousarray(np.asarray(logits, dtype=np.float32))
    tgt = np.asarray(tgt).astype(np.int64)
    sizes = np.ascontiguousarray(np.asarray(sizes, dtype=np.float32))

    flat_logits = logits.reshape(B * T, V)
    flat_tgt = tgt.reshape(B * T)

    in_maps = []
    for cid in range(N_CORES):
        lo = cid * TOK
        shard = flat_logits[lo : lo + TOK]                       # [TOK, V]
        toff = (np.arange(TOK, dtype=np.int64) * V + flat_tgt[lo : lo + TOK])
        toff = toff.astype(np.int32).reshape(NT, P).T.copy()     # [P, NT]
        b = (lo) // T
        assert (lo + TOK - 1) // T == b, "shard must not straddle batch rows"
        in_maps.append(
            {
                "logits": shard,
                "tgt_off": toff,
                "sizes_r": sizes[b].reshape(NB, W),
            }
        )
    return in_maps


def _combine(results):
    nll_total = 0.0
    counts = np.zeros(B, dtype=np.float64)
    for cid, res in enumerate(results):
        o = np.asarray(res["out"], dtype=np.float64)             # [P, 2]
        nll_total += o[:, 0].sum()
        counts[(cid * TOK) // T] += o[:, 1].sum()
    ce = nll_total / (B * T)
    penalty = float(sum(n * (n - 1) / 2 for n in counts))
    return np.float32(ce + ALPHA * penalty)


def run(logits, tgt, sizes, trace=False):
    """Run the SPMD kernel on 8 cores. Returns (output_scalar, exec_time_ns)."""
    from concourse.bass_utils import run_bass_kernel_spmd

    nc = _get_nc()
    in_maps = _make_in_maps(logits, tgt, sizes)
    r = run_bass_kernel_spmd(nc, in_maps, list(range(N_CORES)), trace=trace)
    _NC_CACHE["last_result"] = r
    return _combine(r.results), r.exec_time_ns


def kernel(logits, tgt, sizes):
    out, _ = run(logits, tgt, sizes, trace=False)
    return out


# revision 25
# speedup vs baseline: 1.1217x; 1.1217x over previous
"""Trainium2 Bass kernel for CustomLoss:
    out = mean_{b,t} CE(logits[b,t,:], tgt[b,t]) + penalty
    CE   = logsumexp_V(logits) - logits[tgt]
    penalty = sum_b C(n_b, 2), n_b = #{t : sizes[b, argmax_V logits[b,t,:]] > 0}

Sharding: data-parallel over the 4096 (b,t) tokens -> 512 tokens/core on 8
NeuronCores. Each core streams its [512, 32000] logits shard through SBUF
once at HBM rate; ACT computes exp into a bf16 copy with fused fp32 row-sum
accumulation (logsumexp), DVE computes per-block maxes over the bf16 exp
copy (exp is monotonic, so the argmax block is unchanged) with a chain of
16-bit tensor_tensor MAX folds that run at 2 elem/cycle -- roughly half the
cost of a direct fp32 tensor_reduce, keeping DVE well below the DMA stream
even when the engine clocks are throttled. The winning 256-wide block's
logits and sizes are gathered concurrently per tile; all gather-consuming
ops are deferred until the end of the stream so SWDGE gather latency never
stalls the in-order DVE queue. The final tile ends with a tiny 1024-column
chunk so the post-stream critical path is short. Per-core partial sums
leave as a [128, 2] tile; partitions are summed on host.
"""

from contextlib import ExitStack

import numpy as np

P = 128
V = 32000
B, T = 2, 2048
N_CORES = 8
TOK = (B * T) // N_CORES      # 512 tokens per core
NT = TOK // P                 # 4 token tiles of 128 partitions
W = 256                       # argmax block width
NB = V // W                   # 125 blocks per token row
VC = 8704                     # default vocab chunk (34 blocks)
NBC = VC // W                 # blocks per full chunk
# per-tile (vocab_offset, width) chunk lists; the last tile ends with a
# geometric taper: each chunk's exp (0.83 ns/col) fits under the next
# chunk's DMA transfer (>=1.2 ns/col), so ACT stays caught up to the end
# and the post-stream critical path is short
_FULL = [(0, VC), (VC, VC), (2 * VC, VC), (3 * VC, V - 3 * VC)]
_LAST = _FULL[:3] + [(26112, 2304), (28416, 1536), (29952, 1280),
                     (31232, 768)]
CHUNKS = [_FULL, _FULL, _FULL, _LAST]
MAXCH = max(len(c) for c in CHUNKS)
ALPHA = 1.0

_NC_CACHE = {}


def _build_nc():
    """Build the single-core Bass program (identical on all 8 cores)."""
    import concourse.bacc as bacc
    import concourse.bass as bass
    import concourse.mybir as mybir
    import concourse.tile as tile

    f32 = mybir.dt.float32
    bf16 = mybir.dt.bfloat16
    i32 = mybir.dt.int32
    u32 = mybir.dt.uint32
    AF = mybir.ActivationFunctionType
    ALU = mybir.AluOpType
    AX = mybir.AxisListType

    nc = bacc.Bacc("TRN2", target_bir_lowering=False)
    logits = nc.declare_dram_parameter("logits", [TOK, V], f32, isOutput=False)
    # flat element index t*V + tgt[t], laid out [p, tile] (token = tt*128 + p)
    tgt_off = nc.declare_dram_parameter("tgt_off", [P, NT], i32, isOutput=False)
    sizes_r = nc.declare_dram_parameter("sizes_r", [NB, W], f32, isOutput=False)
    out = nc.declare_dram_parameter("out", [P, 2], f32, isOutput=True)

    with tile.TileContext(nc) as tc, ExitStack() as ctx:
        lp = ctx.enter_context(tc.tile_pool(name="lp", bufs=3))
        ep = ctx.enter_context(tc.tile_pool(name="ep", bufs=3))
        fp = ctx.enter_context(tc.tile_pool(name="fp", bufs=2))
        sm = ctx.enter_context(tc.tile_pool(name="sm", bufs=4))
        ph = ctx.enter_context(tc.tile_pool(name="ph", bufs=2))
        cst = ctx.enter_context(tc.tile_pool(name="cst", bufs=1))

        # ---- persistent constants / accumulators ----
        # row base (flat element index) for each (partition, tile):
        # rb[p, tt] = (tt*P + p) * V  -- exact in f32 (max < 2^24).
        # iota free-axis steps are int16-limited, so one iota per tile column.
        rb_i = cst.tile([P, NT], i32)
        for tt in range(NT):
            nc.gpsimd.iota(
                rb_i[:, tt : tt + 1], pattern=[[1, 1]], base=tt * P * V,
                channel_multiplier=V,
            )
        rb_f = cst.tile([P, NT], f32)
        nc.vector.tensor_copy(rb_f[:], rb_i[:])

        tgt_idx = cst.tile([P, NT], i32)
        # scalar (ACT) HWDGE queue, so the sync queue's first job is chunk 0
        nc.scalar.dma_start(tgt_idx[:], tgt_off[:, :])
        tgt_logit = cst.tile([P, NT], f32)
        for tt in range(NT):
            nc.gpsimd.indirect_dma_start(
                out=tgt_logit[:, tt : tt + 1],
                out_offset=None,
                in_=logits[:, :],
                in_offset=bass.IndirectOffsetOnAxis(
                    ap=tgt_idx[:, tt : tt + 1], axis=1
                ),
            )

        tot_cols = cst.tile([P, NT], f32)   # per-tile sum(exp) totals
        m_cols = cst.tile([P, NT], f32)     # per-tile positive-size indicator
        acc = cst.tile([P, 2], f32)
        # per-tile gather landing zones (consumed late, so the gathers'
        # HBM latency never stalls the in-order DVE queue mid-stream)
        szb_all = cst.tile([P, NT * W], f32)
        blk_all = cst.tile([P, NT * W], f32)

        def do_chunk(tt, bmax, sexp, c, off, vc):
            nb = vc // W
            lt = lp.tile([P, VC], f32, tag="lt")
            nc.sync.dma_start(
                lt[:, :vc], logits[tt * P : (tt + 1) * P, off : off + vc]
            )
            et = ep.tile([P, VC], bf16, tag="et")
            nc.scalar.activation(
                et[:, :vc], lt[:, :vc], AF.Exp, accum_out=sexp[:, c : c + 1]
            )
            # block-max over the bf16 exp copy: three 2x-rate TT-max folds
            # [P,nb,256] -> [P,nb,32], then a small 1x reduce to [P,nb].
            # Tiny chunks skip the folds: 4 instruction overheads cost more
            # wall time than a direct 1x reduce below ~1.5k columns.
            e3 = et[:, :vc].rearrange("p (b w) -> p b w", w=W)
            if vc <= 1536:
                nc.vector.tensor_reduce(
                    bmax[:, off // W : (off + vc) // W], e3, axis=AX.X,
                    op=ALU.max,
                )
                return
            f1 = fp.tile([P, NBC * 128], bf16, tag="f1")
            f13 = f1[:, : nb * 128].rearrange("p (b w) -> p b w", w=128)
            nc.vector.tensor_tensor(
                f13, e3[:, :, 0:128], e3[:, :, 128:256], op=ALU.max
            )
            f2 = fp.tile([P, NBC * 64], bf16, tag="f2")
            f23 = f2[:, : nb * 64].rearrange("p (b w) -> p b w", w=64)
            nc.vector.tensor_tensor(
                f23, f13[:, :, 0:64], f13[:, :, 64:128], op=ALU.max
            )
            f3 = fp.tile([P, NBC * 32], bf16, tag="f3")
            f33 = f3[:, : nb * 32].rearrange("p (b w) -> p b w", w=32)
            nc.vector.tensor_tensor(
                f33, f23[:, :, 0:32], f23[:, :, 32:64], op=ALU.max
            )
            nc.vector.tensor_reduce(
                bmax[:, off // W : (off + vc) // W], f33, axis=AX.X, op=ALU.max
            )

        def select_tile(tt, bmax, sexp, nch):
            """Pick the winning block and *issue* its two gathers (nothing
            here reads gathered data, so no gather-latency DVE stalls)."""
            top8 = sm.tile([P, 8], bf16, tag="top8")
            nc.vector.max(top8[:], bmax[:])
            bix8 = sm.tile([P, 8], u32, tag="bix8")
            nc.vector.max_index(bix8[:], top8[:], bmax[:])
            # logits-block gather first (its consumer chain is the longer
            # pole in the tail): flat elem idx = rb + bid*W, exact in f32
            bidf = sm.tile([P, 1], f32, tag="bidf")
            nc.vector.tensor_copy(bidf[:], bix8[:, 0:1])
            gsf = sm.tile([P, 1], f32, tag="gsf")
            nc.vector.tensor_scalar(
                gsf[:], bidf[:], float(W), rb_f[:, tt : tt + 1],
                op0=ALU.mult, op1=ALU.add,
            )
            gsi = sm.tile([P, 1], i32, tag="gsi")
            nc.vector.tensor_copy(gsi[:], gsf[:])
            nc.gpsimd.indirect_dma_start(
                out=blk_all[:, tt * W : (tt + 1) * W],
                out_offset=None,
                in_=logits[:, :],
                in_offset=bass.IndirectOffsetOnAxis(ap=gsi[:, 0:1], axis=1),
            )
            bid_i = sm.tile([P, 1], i32, tag="bid_i")
            nc.vector.tensor_copy(bid_i[:], bix8[:, 0:1])
            nc.gpsimd.indirect_dma_start(
                out=szb_all[:, tt * W : (tt + 1) * W],
                out_offset=None,
                in_=sizes_r[:, :],
                in_offset=bass.IndirectOffsetOnAxis(ap=bid_i[:, 0:1], axis=0),
            )
            # off the argmax critical path, so last: per-tile exp total
            nc.vector.reduce_sum(
                tot_cols[:, tt : tt + 1], sexp[:, :nch], axis=AX.X
            )

        def finish_tile(tt):
            """Consume the gathered blocks: size-positivity at the argmax.
            The one-hot comes from comparing the block against its own max
            (values are distinct floats), so no index math is needed."""
            blk = blk_all[:, tt * W : (tt + 1) * W]
            szb = szb_all[:, tt * W : (tt + 1) * W]
            # blk was gathered first, so consume it first
            blk8 = sm.tile([P, 8], f32, tag="blk8")
            nc.vector.max(blk8[:], blk)
            oh = ph.tile([P, W], f32, tag="oh")
            nc.vector.tensor_scalar(
                oh[:], blk, blk8[:, 0:1], None, op0=ALU.is_equal
            )
            mb = ph.tile([P, W], f32, tag="mb")
            nc.vector.tensor_scalar(mb[:], szb, 0.0, None, op0=ALU.is_gt)
            prod = ph.tile([P, W], f32, tag="prod")
            nc.vector.tensor_tensor(prod[:], oh[:], mb[:], op=ALU.mult)
            nc.vector.tensor_reduce(
                m_cols[:, tt : tt + 1], prod[:], axis=AX.X, op=ALU.add
            )

        for tt in range(NT - 1):
            bmax = sm.tile([P, NB], bf16, tag="bmax")
            sexp = sm.tile([P, MAXCH], f32, tag="sexp")
            for c, (off, vc) in enumerate(CHUNKS[tt]):
                do_chunk(tt, bmax, sexp, c, off, vc)
            select_tile(tt, bmax, sexp, len(CHUNKS[tt]))
        # last tile: stream its big chunks, slot the earlier tiles' deferred
        # work in while its data is still in flight, then its tiny tail
        # chunk, so the post-stream critical path is short
        last = NT - 1
        chunks = CHUNKS[last]
        bmax = sm.tile([P, NB], bf16, tag="bmax")
        sexp = sm.tile([P, MAXCH], f32, tag="sexp")
        for c, (off, vc) in enumerate(chunks[:3]):
            do_chunk(last, bmax, sexp, c, off, vc)
        for tt in range(NT - 1):
            finish_tile(tt)
        for c, (off, vc) in enumerate(chunks[3:], start=3):
            do_chunk(last, bmax, sexp, c, off, vc)
        select_tile(last, bmax, sexp, len(chunks))
        finish_tile(last)

        # ---- nll, batched: one Ln activation for all tiles ----
        lse_cols = cst.tile([P, NT], f32)
        nc.scalar.activation(lse_cols[:], tot_cols[:], AF.Ln)
        nll_cols = cst.tile([P, NT], f32)
        nc.vector.tensor_tensor(
            nll_cols[:], lse_cols[:], tgt_logit[:], op=ALU.subtract
        )
        nc.vector.reduce_sum(acc[:, 0:1], nll_cols[:], axis=AX.X)
        nc.vector.reduce_sum(acc[:, 1:2], m_cols[:], axis=AX.X)
        nc.sync.dma_start(out[:, :], acc[:])

    nc.finalize()
    return nc


def _get_nc():
    if "nc" not in _NC_CACHE:
        _NC_CACHE["nc"] = _build_nc()
    return _NC_CACHE["nc"]


def _make_in_maps(logits, tgt, sizes):
    logits = np.ascontiguousarray(np.asarray(logits, dtype=np.float32))
    tgt = np.asarray(tgt).astype(np.int64)
    sizes = np.ascontiguousarray(np.asarray(sizes, dtype=np.float32))

    flat_logits = logits.reshape(B * T, V)
    flat_tgt = tgt.reshape(B * T)

    in_maps = []
    for cid in range(N_CORES):
        lo = cid * TOK
        shard = flat_logits[lo : lo + TOK]                       # [TOK, V]
        toff = (np.arange(TOK, dtype=np.int64) * V + flat_tgt[lo : lo + TOK])
        toff = toff.astype(np.int32).reshape(NT, P).T.copy()     # [P, NT]
        b = (lo) // T
        assert (lo + TOK - 1) // T == b, "shard must not straddle batch rows"
        in_maps.append(
            {
                "logits": shard,
                "tgt_off": toff,
                "sizes_r": sizes[b].reshape(NB, W),
            }
        )
    return in_maps


def _combine(results):
    nll_total = 0.0
    counts = np.zeros(B, dtype=np.float64)
    for cid, res in enumerate(results):
        o = np.asarray(res["out"], dtype=np.float64)             # [P, 2]
        nll_total += o[:, 0].sum()
        counts[(cid * TOK) // T] += o[:, 1].sum()
    ce = nll_total / (B * T)
    penalty = float(sum(n * (n - 1) / 2 for n in counts))
    return np.float32(ce + ALPHA * penalty)


def run(logits, tgt, sizes, trace=False):
    """Run the SPMD kernel on 8 cores. Returns (output_scalar, exec_time_ns)."""
    from concourse.bass_utils import run_bass_kernel_spmd

    nc = _get_nc()
    in_maps = _make_in_maps(logits, tgt, sizes)
    r = run_bass_kernel_spmd(nc, in_maps, list(range(N_CORES)), trace=trace)
    _NC_CACHE["last_result"] = r
    return _combine(r.results), r.exec_time_ns


def kernel(logits, tgt, sizes):
    out, _ = run(logits, tgt, sizes, trace=False)
    return out
